# revision 2
# baseline (speedup 1.0000x reference)
"""Trainium2 Bass kernel for the Clifford-algebra geometric product.

  out[..., j] = sum_{i,k} a[..., i] * cayley[i, j, k] * b[..., k]

Full inputs a, b: (2048, 1024, 8) fp32, cayley: (8, 8, 8) fp32.
Sharding: pure data parallelism over the leading batch axis across 8
NeuronCores; the cayley table's nonzero structure is folded into the
instruction stream (immediates), so it needs no on-device storage.

Per-core layout: the local (256*1024, 8) position-major array is viewed as
[128 partitions, 2048*8 free] (position-major split across partitions).
For each tile of W positions/partition:
  - 64 scalar_tensor_tensor ops compute signed products
      p0[:, j*8+l, :] = (a_i * sign) * b_k      for term l of output blade j
  - 3 full-width tensor_tensor adds do the binary-tree reduction; the last
    level writes with a transposed access pattern directly into the
    natural (position, blade) output tile.
"""

import sys

if "/opt/trn_rl_repo" not in sys.path:
    sys.path.insert(0, "/opt/trn_rl_repo")

import numpy as np

N_CORES = 8
P = 128  # SBUF partitions
N = 8    # blades
W = 256  # positions per partition per tile

_module_cache = {}


def _terms_by_j(cayley: np.ndarray):
    """Group the nonzero cayley entries by output blade j."""
    terms = [[] for _ in range(N)]
    for i in range(N):
        for j in range(N):
            for k in range(N):
                v = float(cayley[i, j, k])
                if v != 0.0:
                    terms[j].append((i, k, v))
    return terms


def _build_module(npos_local: int, terms):
    import concourse.bacc as bacc
    import concourse.mybir as mybir
    import concourse.tile as tile

    assert npos_local % (P * W) == 0
    T = npos_local // (P * W)
    fast = all(len(t) == 8 for t in terms)

    nc = bacc.Bacc(None, target_bir_lowering=False, debug=False)
    with tile.TileContext(nc) as tc:
        with tc.tile_pool(name="dram", bufs=1, space="DRAM") as dram:
            a = dram.tile((npos_local, N), mybir.dt.float32, kind="ExternalInput")
            b = dram.tile((npos_local, N), mybir.dt.float32, kind="ExternalInput")
            out = dram.tile((npos_local, N), mybir.dt.float32, kind="ExternalOutput")
            av = a[:].rearrange("(p f) n -> p (f n)", p=P)
            bv = b[:].rearrange("(p f) n -> p (f n)", p=P)
            ov = out[:].rearrange("(p f) n -> p (f n)", p=P)
            with (
                tc.tile_pool(name="io", bufs=2) as io_pool,
                tc.tile_pool(name="prod", bufs=1) as prod_pool,
            ):
                for t in range(T):
                    sl = slice(t * W * N, (t + 1) * W * N)
                    ta = io_pool.tile([P, W, N], mybir.dt.float32, tag="ta")
                    tb = io_pool.tile([P, W, N], mybir.dt.float32, tag="tb")
                    to = io_pool.tile([P, W, N], mybir.dt.float32, tag="to")
                    nc.sync.dma_start(
                        out=ta[:].rearrange("p f n -> p (f n)"), in_=av[:, sl]
                    )
                    nc.sync.dma_start(
                        out=tb[:].rearrange("p f n -> p (f n)"), in_=bv[:, sl]
                    )
                    if fast:
                        p0 = prod_pool.tile([P, 64, W], mybir.dt.float32, tag="p0")
                        p1 = prod_pool.tile([P, 32, W], mybir.dt.float32, tag="p1")
                        p2 = prod_pool.tile([P, 16, W], mybir.dt.float32, tag="p2")
                        for j in range(N):
                            for l, (i, k, v) in enumerate(terms[j]):
                                nc.vector.scalar_tensor_tensor(
                                    out=p0[:, j * 8 + l, :],
                                    in0=ta[:, :, i],
                                    scalar=v,
                                    in1=tb[:, :, k],
                                    op0=mybir.AluOpType.mult,
                                    op1=mybir.AluOpType.mult,
                                )
                        nc.vector.tensor_tensor(
                            out=p1[:], in0=p0[:, 0::2, :], in1=p0[:, 1::2, :],
                            op=mybir.AluOpType.add,
                        )
                        nc.vector.tensor_tensor(
                            out=p2[:], in0=p1[:, 0::2, :], in1=p1[:, 1::2, :],
                            op=mybir.AluOpType.add,
                        )
                        nc.vector.tensor_tensor(
                            out=to[:].transpose([0, 2, 1]),
                            in0=p2[:, 0::2, :], in1=p2[:, 1::2, :],
                            op=mybir.AluOpType.add,
                        )
                    else:
                        # generic fallback: per-j product + sequential adds
                        pa = prod_pool.tile([P, W], mybir.dt.float32, tag="pa")
                        acc = prod_pool.tile([P, W], mybir.dt.float32, tag="acc")
                        for j in range(N):
                            if not terms[j]:
                                nc.vector.memset(to[:, :, j], 0.0)
                                continue
                            i, k, v = terms[j][0]
                            nc.vector.scalar_tensor_tensor(
                                out=acc[:], in0=ta[:, :, i], scalar=v,
                                in1=tb[:, :, k],
                                op0=mybir.AluOpType.mult, op1=mybir.AluOpType.mult,
                            )
                            for (i, k, v) in terms[j][1:]:
                                nc.vector.scalar_tensor_tensor(
                                    out=pa[:], in0=ta[:, :, i], scalar=v,
                                    in1=tb[:, :, k],
                                    op0=mybir.AluOpType.mult, op1=mybir.AluOpType.mult,
                                )
                                nc.vector.tensor_tensor(
                                    out=acc[:], in0=acc[:], in1=pa[:],
                                    op=mybir.AluOpType.add,
                                )
                            nc.vector.tensor_copy(out=to[:, :, j], in_=acc[:])
                    nc.sync.dma_start(
                        out=ov[:, sl], in_=to[:].rearrange("p f n -> p (f n)")
                    )
    nc.compile()
    return nc, a.name, b.name, out.name


W_V2 = 256
GP_COLS = 64


def _get_module(npos_local: int, cayley: np.ndarray):
    key = (npos_local, cayley.tobytes())
    if key not in _module_cache:
        import kernel2

        plan = kernel2.build_plan(cayley)
        if plan is not None and npos_local % (P * W_V2) == 0:
            _module_cache[key] = kernel2.build_module(
                npos_local, plan, W=W_V2, gp_cols=GP_COLS
            )
        else:
            _module_cache[key] = _build_module(npos_local, _terms_by_j(cayley))
    return _module_cache[key]


def _run(inputs: dict, trace: bool = False, tmpdir=None):
    a = np.asarray(inputs["a"], dtype=np.float32)
    b = np.asarray(inputs["b"], dtype=np.float32)
    cayley = np.asarray(inputs["cayley"], dtype=np.float32)
    B, S, NN = a.shape
    assert NN == N and b.shape == a.shape and cayley.shape == (N, N, N)
    assert B % N_CORES == 0
    nb = B // N_CORES
    npos_local = nb * S

    nc, a_name, b_name, out_name = _get_module(npos_local, cayley)

    a_sh = a.reshape(N_CORES, npos_local, N)
    b_sh = b.reshape(N_CORES, npos_local, N)
    in_maps = [
        {a_name: np.ascontiguousarray(a_sh[c]), b_name: np.ascontiguousarray(b_sh[c])}
        for c in range(N_CORES)
    ]

    from concourse import bass_utils

    kwargs = {}
    if trace:
        _install_ntff_shim()
        bass_utils.upload_artifacts = lambda d: f"local:{d}"
        kwargs = {"trace": True, "tmpdir": tmpdir}
    res = bass_utils.run_bass_kernel_spmd(
        nc, in_maps, core_ids=list(range(N_CORES)), **kwargs
    )
    out = np.concatenate(
        [res.results[c][out_name].reshape(1, nb, S, N) for c in range(N_CORES)], axis=0
    ).reshape(B, S, N)
    return out, res


def kernel(**inputs) -> np.ndarray:
    out, _ = _run(inputs, trace=False)
    return out


def kernel_traced(**inputs):
    """Run with NTFF profiling; returns (out, exec_time_ns, trace_path)."""
    import tempfile

    out, res = _run(inputs, trace=True, tmpdir=tempfile.mkdtemp(prefix="gp_trace_"))
    trace_path = res.instructions_and_trace[1] if res.instructions_and_trace else None
    return out, res.exec_time_ns, trace_path


def _install_ntff_shim():
    """Provide antenv.axon_hooks with an NTFF profile hook if missing."""
    try:
        from antenv.axon_hooks import get_axon_ntff_profile_hook  # noqa: F401

        return
    except ImportError:
        pass
    import types, ctypes, contextlib

    holder = {"hook": None}
    mod = types.ModuleType("antenv.axon_hooks")
    mod.set_axon_ntff_profile_hook = lambda h: holder.__setitem__("hook", h)
    mod.get_axon_ntff_profile_hook = lambda: holder["hook"]
    sys.modules["antenv.axon_hooks"] = mod

    so_path = "/opt/axon/libaxon_pjrt.so"
    try:
        lib = ctypes.CDLL(so_path)
        if not hasattr(lib, "axon_start_nrt_profile"):
            return
    except OSError:
        return
    lib.axon_start_nrt_profile.argtypes = [
        ctypes.POINTER(ctypes.c_int64),
        ctypes.c_size_t,
    ]
    lib.axon_start_nrt_profile.restype = ctypes.c_int64
    lib.axon_stop_nrt_profile.argtypes = [ctypes.c_char_p]
    lib.axon_stop_nrt_profile.restype = ctypes.c_int64

    @contextlib.contextmanager
    def _hook(output_dir, device_ids):
        import jax

        jax.devices()
        if device_ids:
            ids = (ctypes.c_int64 * len(device_ids))(*device_ids)
            rc = lib.axon_start_nrt_profile(ids, len(device_ids))
        else:
            rc = lib.axon_start_nrt_profile(None, 0)
        if rc != 0:
            raise RuntimeError(f"axon_start_nrt_profile rc={rc}")
        try:
            yield
        finally:
            n = lib.axon_stop_nrt_profile(str(output_dir).encode())
            print(f"profile: {n} file(s) written to {output_dir}", file=sys.stderr)

    mod.set_axon_ntff_profile_hook(_hook)


# revision 4
# speedup vs baseline: 1.5865x; 1.5865x over previous
"""Trainium2 Bass kernel for the Clifford-algebra geometric product.

  out[..., j] = sum_{i,k} a[..., i] * cayley[i, j, k] * b[..., k]

Full inputs a, b: (2048, 1024, 8) fp32, cayley: (8, 8, 8) fp32.
Sharding: pure data parallelism over the leading batch axis across 8
NeuronCores; the cayley table's nonzero structure is folded into the
instruction stream (immediates), so it needs no on-device storage.

Per-core layout: the local (256*1024, 8) position-major array is viewed as
[128 partitions, 2048*8 free] (position-major split across partitions).
For each tile of W positions/partition:
  - 64 scalar_tensor_tensor ops compute signed products
      p0[:, j*8+l, :] = (a_i * sign) * b_k      for term l of output blade j
  - 3 full-width tensor_tensor adds do the binary-tree reduction; the last
    level writes with a transposed access pattern directly into the
    natural (position, blade) output tile.
"""

import sys

if "/opt/trn_rl_repo" not in sys.path:
    sys.path.insert(0, "/opt/trn_rl_repo")

import numpy as np

N_CORES = 8
P = 128  # SBUF partitions
N = 8    # blades
W = 256  # positions per partition per tile

_module_cache = {}


def _terms_by_j(cayley: np.ndarray):
    """Group the nonzero cayley entries by output blade j."""
    terms = [[] for _ in range(N)]
    for i in range(N):
        for j in range(N):
            for k in range(N):
                v = float(cayley[i, j, k])
                if v != 0.0:
                    terms[j].append((i, k, v))
    return terms


def _build_module(npos_local: int, terms):
    import concourse.bacc as bacc
    import concourse.mybir as mybir
    import concourse.tile as tile

    assert npos_local % (P * W) == 0
    T = npos_local // (P * W)
    fast = all(len(t) == 8 for t in terms)

    nc = bacc.Bacc(None, target_bir_lowering=False, debug=False)
    with tile.TileContext(nc) as tc:
        with tc.tile_pool(name="dram", bufs=1, space="DRAM") as dram:
            a = dram.tile((npos_local, N), mybir.dt.float32, kind="ExternalInput")
            b = dram.tile((npos_local, N), mybir.dt.float32, kind="ExternalInput")
            out = dram.tile((npos_local, N), mybir.dt.float32, kind="ExternalOutput")
            av = a[:].rearrange("(p f) n -> p (f n)", p=P)
            bv = b[:].rearrange("(p f) n -> p (f n)", p=P)
            ov = out[:].rearrange("(p f) n -> p (f n)", p=P)
            with (
                tc.tile_pool(name="io", bufs=2) as io_pool,
                tc.tile_pool(name="prod", bufs=1) as prod_pool,
            ):
                for t in range(T):
                    sl = slice(t * W * N, (t + 1) * W * N)
                    ta = io_pool.tile([P, W, N], mybir.dt.float32, tag="ta")
                    tb = io_pool.tile([P, W, N], mybir.dt.float32, tag="tb")
                    to = io_pool.tile([P, W, N], mybir.dt.float32, tag="to")
                    nc.sync.dma_start(
                        out=ta[:].rearrange("p f n -> p (f n)"), in_=av[:, sl]
                    )
                    nc.sync.dma_start(
                        out=tb[:].rearrange("p f n -> p (f n)"), in_=bv[:, sl]
                    )
                    if fast:
                        p0 = prod_pool.tile([P, 64, W], mybir.dt.float32, tag="p0")
                        p1 = prod_pool.tile([P, 32, W], mybir.dt.float32, tag="p1")
                        p2 = prod_pool.tile([P, 16, W], mybir.dt.float32, tag="p2")
                        for j in range(N):
                            for l, (i, k, v) in enumerate(terms[j]):
                                nc.vector.scalar_tensor_tensor(
                                    out=p0[:, j * 8 + l, :],
                                    in0=ta[:, :, i],
                                    scalar=v,
                                    in1=tb[:, :, k],
                                    op0=mybir.AluOpType.mult,
                                    op1=mybir.AluOpType.mult,
                                )
                        nc.vector.tensor_tensor(
                            out=p1[:], in0=p0[:, 0::2, :], in1=p0[:, 1::2, :],
                            op=mybir.AluOpType.add,
                        )
                        nc.vector.tensor_tensor(
                            out=p2[:], in0=p1[:, 0::2, :], in1=p1[:, 1::2, :],
                            op=mybir.AluOpType.add,
                        )
                        nc.vector.tensor_tensor(
                            out=to[:].transpose([0, 2, 1]),
                            in0=p2[:, 0::2, :], in1=p2[:, 1::2, :],
                            op=mybir.AluOpType.add,
                        )
                    else:
                        # generic fallback: per-j product + sequential adds
                        pa = prod_pool.tile([P, W], mybir.dt.float32, tag="pa")
                        acc = prod_pool.tile([P, W], mybir.dt.float32, tag="acc")
                        for j in range(N):
                            if not terms[j]:
                                nc.vector.memset(to[:, :, j], 0.0)
                                continue
                            i, k, v = terms[j][0]
                            nc.vector.scalar_tensor_tensor(
                                out=acc[:], in0=ta[:, :, i], scalar=v,
                                in1=tb[:, :, k],
                                op0=mybir.AluOpType.mult, op1=mybir.AluOpType.mult,
                            )
                            for (i, k, v) in terms[j][1:]:
                                nc.vector.scalar_tensor_tensor(
                                    out=pa[:], in0=ta[:, :, i], scalar=v,
                                    in1=tb[:, :, k],
                                    op0=mybir.AluOpType.mult, op1=mybir.AluOpType.mult,
                                )
                                nc.vector.tensor_tensor(
                                    out=acc[:], in0=acc[:], in1=pa[:],
                                    op=mybir.AluOpType.add,
                                )
                            nc.vector.tensor_copy(out=to[:, :, j], in_=acc[:])
                    nc.sync.dma_start(
                        out=ov[:, sl], in_=to[:].rearrange("p f n -> p (f n)")
                    )
    nc.compile()
    return nc, a.name, b.name, out.name


W_V2 = 256
GP_COLS = 0


def _get_module(npos_local: int, cayley: np.ndarray):
    key = (npos_local, cayley.tobytes())
    if key not in _module_cache:
        import kernel2

        plan = kernel2.build_plan(cayley, max_digits=1)
        if plan is not None and npos_local % (P * W_V2) == 0:
            _module_cache[key] = kernel2.build_module_planes(
                npos_local, plan, W=W_V2, gp_cols=GP_COLS
            )
        else:
            _module_cache[key] = _build_module(npos_local, _terms_by_j(cayley))
    return _module_cache[key]


def _run(inputs: dict, trace: bool = False, tmpdir=None):
    a = np.asarray(inputs["a"], dtype=np.float32)
    b = np.asarray(inputs["b"], dtype=np.float32)
    cayley = np.asarray(inputs["cayley"], dtype=np.float32)
    B, S, NN = a.shape
    assert NN == N and b.shape == a.shape and cayley.shape == (N, N, N)
    assert B % N_CORES == 0
    nb = B // N_CORES
    npos_local = nb * S

    nc, a_name, b_name, out_name = _get_module(npos_local, cayley)

    a_sh = a.reshape(N_CORES, npos_local, N)
    b_sh = b.reshape(N_CORES, npos_local, N)
    in_maps = [
        {a_name: np.ascontiguousarray(a_sh[c]), b_name: np.ascontiguousarray(b_sh[c])}
        for c in range(N_CORES)
    ]

    from concourse import bass_utils

    kwargs = {}
    if trace:
        _install_ntff_shim()
        bass_utils.upload_artifacts = lambda d: f"local:{d}"
        kwargs = {"trace": True, "tmpdir": tmpdir}
    res = bass_utils.run_bass_kernel_spmd(
        nc, in_maps, core_ids=list(range(N_CORES)), **kwargs
    )
    out = np.concatenate(
        [res.results[c][out_name].reshape(1, nb, S, N) for c in range(N_CORES)], axis=0
    ).reshape(B, S, N)
    return out, res


def kernel(**inputs) -> np.ndarray:
    out, _ = _run(inputs, trace=False)
    return out


def kernel_traced(**inputs):
    """Run with NTFF profiling; returns (out, exec_time_ns, trace_path)."""
    import tempfile

    out, res = _run(inputs, trace=True, tmpdir=tempfile.mkdtemp(prefix="gp_trace_"))
    trace_path = res.instructions_and_trace[1] if res.instructions_and_trace else None
    return out, res.exec_time_ns, trace_path


def _install_ntff_shim():
    """Provide antenv.axon_hooks with an NTFF profile hook if missing."""
    try:
        from antenv.axon_hooks import get_axon_ntff_profile_hook  # noqa: F401

        return
    except ImportError:
        pass
    import types, ctypes, contextlib

    holder = {"hook": None}
    mod = types.ModuleType("antenv.axon_hooks")
    mod.set_axon_ntff_profile_hook = lambda h: holder.__setitem__("hook", h)
    mod.get_axon_ntff_profile_hook = lambda: holder["hook"]
    sys.modules["antenv.axon_hooks"] = mod

    so_path = "/opt/axon/libaxon_pjrt.so"
    try:
        lib = ctypes.CDLL(so_path)
        if not hasattr(lib, "axon_start_nrt_profile"):
            return
    except OSError:
        return
    lib.axon_start_nrt_profile.argtypes = [
        ctypes.POINTER(ctypes.c_int64),
        ctypes.c_size_t,
    ]
    lib.axon_start_nrt_profile.restype = ctypes.c_int64
    lib.axon_stop_nrt_profile.argtypes = [ctypes.c_char_p]
    lib.axon_stop_nrt_profile.restype = ctypes.c_int64

    @contextlib.contextmanager
    def _hook(output_dir, device_ids):
        import jax

        jax.devices()
        if device_ids:
            ids = (ctypes.c_int64 * len(device_ids))(*device_ids)
            rc = lib.axon_start_nrt_profile(ids, len(device_ids))
        else:
            rc = lib.axon_start_nrt_profile(None, 0)
        if rc != 0:
            raise RuntimeError(f"axon_start_nrt_profile rc={rc}")
        try:
            yield
        finally:
            n = lib.axon_stop_nrt_profile(str(output_dir).encode())
            print(f"profile: {n} file(s) written to {output_dir}", file=sys.stderr)

    mod.set_axon_ntff_profile_hook(_hook)
